# revision 8
# baseline (speedup 1.0000x reference)
"""CLIP (InfoNCE) loss kernel for Trainium2, 8 NeuronCores.

loss = 0.5*(ce_m + ce_s) where
  ce_m = mean_i( LSE_j(l[i,:]) - l[i,i] ),  ce_s = mean_j( LSE_i(l[:,j]) - l[j,j] )
  l = logit_scale * (m @ s.T),  B=16384, D=256.

Strategy (data parallel on batch rows, 8 cores):
  - core c owns rows [c*2048, (c+1)*2048) of m; gets the FULL s.
  - host pre-scales m by logit_scale and pre-transposes both operands to
    [D, rows] bf16 so they feed the PE directly (lhsT / rhs layout).
  - per core: 16 m-tiles x 32 s-panels of [128, 512] logits in PSUM (f32
    accumulation over K=256 in two 128-chunks).
  - one ScalarE activation computes E = exp(l - SHIFT) (bf16) AND the fused
    per-row partial sums via accum_out (f32, on the un-rounded values).
  - a ones-vector matmul (lhsT=[128,1]) accumulates per-column sums of E in
    PSUM across the 16 m-tiles of a panel -> col partial sums per core.
  - diag l[i,i] is computed exactly in f32 as a fused row-dot of the
    natural-layout shards.
  - host merges tiny per-core partials in float64:
      rowLSE = SHIFT + log(rowsum); colLSE = SHIFT + log(sum_c colsum_c)
      loss = mean(0.5*(rowLSE + colLSE) - diag)

SHIFT is a single global logsumexp shift. logits ~ N(0, (scale*sqrt(D))^2);
their max over B^2 samples is ~6 sigma, so SHIFT = 6*|scale|*sqrt(D) keeps
exp() in [e^-180, e^~5]: no overflow, and anything that underflows f32 is
~e^-80 below the column max, i.e. far below f32 relative precision anyway.
"""

import math
from contextlib import ExitStack

import numpy as np
import ml_dtypes

import concourse.bacc as bacc
import concourse.bass as bass
import concourse.tile as tile
from concourse import mybir
from concourse.bass_utils import run_bass_kernel_spmd

BF16 = ml_dtypes.bfloat16

B = 16384
D = 256
NCORES = 8
ROWS = B // NCORES          # 2048 rows per core
P = 128                     # partitions
MT = ROWS // P              # 16 m-tiles per core
PN = 512                    # panel width (psum bank: 512 f32)
NT = B // PN                # 32 panels
KC = D // P                 # 2 contraction chunks
NQ = 4                      # sT column quarters (DMA chunking for early start)
QW = B // NQ

_nc_cache: dict[float, "bass.Bass"] = {}

f32 = mybir.dt.float32
bf16 = mybir.dt.bfloat16


def _build(shift: float) -> "bass.Bass":
    nc = bacc.Bacc(trn_type="TRN2")

    mT_d = nc.dram_tensor("mT", [D, ROWS], bf16, kind="ExternalInput")
    sT_d = nc.dram_tensor("sT", [D, B], bf16, kind="ExternalInput")
    mnat_d = nc.dram_tensor("mnat", [ROWS, D], f32, kind="ExternalInput")
    snat_d = nc.dram_tensor("snat", [ROWS, D], f32, kind="ExternalInput")

    rowsum_d = nc.dram_tensor("rowsum", [P, MT], f32, kind="ExternalOutput")
    diag_d = nc.dram_tensor("diag", [P, MT], f32, kind="ExternalOutput")
    colsum_d = nc.dram_tensor("colsum", [1, B], f32, kind="ExternalOutput")

    with ExitStack() as ctx:
        tc = ctx.enter_context(tile.TileContext(nc))
        singles = ctx.enter_context(tc.tile_pool(name="singles", bufs=1))
        epool = ctx.enter_context(tc.tile_pool(name="epool", bufs=6))
        diagpool = ctx.enter_context(tc.tile_pool(name="diagpool", bufs=4))
        mainps = ctx.enter_context(tc.tile_pool(name="mainps", bufs=4, space="PSUM"))
        colps = ctx.enter_context(tc.tile_pool(name="colps", bufs=2, space="PSUM"))

        # ---- persistent SBUF ----
        # full s^T, as [k-chunk][quarter] tiles so matmuls can start as soon
        # as their quarter has landed.
        sT_sb = [
            [
                singles.tile(
                    [P, QW], bf16, name=f"sT_{k}_{q}", tag=f"sT_{k}_{q}"
                )
                for q in range(NQ)
            ]
            for k in range(KC)
        ]
        for k in range(KC):
            for q in range(NQ):
                nc.sync.dma_start(
                    out=sT_sb[k][q],
                    in_=sT_d[k * P : (k + 1) * P, q * QW : (q + 1) * QW],
                )
        mT_sb = singles.tile([P, KC, ROWS], bf16, tag="mT")
        for k in range(KC):
            nc.sync.dma_start(out=mT_sb[:, k, :], in_=mT_d[k * P : (k + 1) * P, :])

        ones = singles.tile([P, 1], bf16, tag="ones")
        nc.vector.memset(ones, 1.0)
        negshift = singles.tile([P, 1], f32, tag="negshift")
        nc.vector.memset(negshift, -shift)

        rowsums_sb = singles.tile([P, MT * NT], f32, tag="rowsums")
        rowfinal = singles.tile([P, MT], f32, tag="rowfinal")
        colsum_sb = singles.tile([1, B], f32, tag="colsum")
        diagfinal = singles.tile([P, MT], f32, tag="diagfinal")

        # ---- diag: exact f32 row-dot of the natural-layout shards ----
        for mt in range(MT):
            mn = diagpool.tile([P, D], f32, tag="mn")
            sn = diagpool.tile([P, D], f32, tag="sn")
            prod = diagpool.tile([P, D], f32, tag="prod")
            nc.sync.dma_start(out=mn, in_=mnat_d[mt * P : (mt + 1) * P, :])
            nc.sync.dma_start(out=sn, in_=snat_d[mt * P : (mt + 1) * P, :])
            # (tensor_tensor_reduce compiles but faults on this HW/runtime
            # combo — use plain mul + reduce instead)
            nc.vector.tensor_mul(prod, mn, sn)
            nc.vector.reduce_sum(
                diagfinal[:, mt : mt + 1], prod, axis=mybir.AxisListType.X
            )

        # ---- main sweep ----
        for nt in range(NT):
            q, j = divmod(nt, QW // PN)
            colpsum = colps.tile([1, PN], f32)
            for mt in range(MT):
                ps = mainps.tile([P, PN], f32)
                for k in range(KC):
                    nc.tensor.matmul(
                        ps,
                        lhsT=mT_sb[:, k, mt * P : (mt + 1) * P],
                        rhs=sT_sb[k][q][:, j * PN : (j + 1) * PN],
                        start=(k == 0),
                        stop=(k == KC - 1),
                    )
                e = epool.tile([P, PN], bf16)
                slot = mt * NT + nt
                nc.scalar.activation(
                    e,
                    ps,
                    mybir.ActivationFunctionType.Exp,
                    bias=negshift[:, 0:1],
                    scale=1.0,
                    accum_out=rowsums_sb[:, slot : slot + 1],
                )
                nc.tensor.matmul(
                    colpsum,
                    lhsT=ones,
                    rhs=e,
                    start=(mt == 0),
                    stop=(mt == MT - 1),
                )
            nc.vector.tensor_copy(
                out=colsum_sb[:, nt * PN : (nt + 1) * PN], in_=colpsum
            )

        for mt in range(MT):
            nc.vector.reduce_sum(
                rowfinal[:, mt : mt + 1],
                rowsums_sb[:, mt * NT : (mt + 1) * NT],
                axis=mybir.AxisListType.X,
            )

        nc.sync.dma_start(out=rowsum_d[:, :], in_=rowfinal)
        nc.sync.dma_start(out=diag_d[:, :], in_=diagfinal)
        nc.sync.dma_start(out=colsum_d[:, :], in_=colsum_sb)

    nc.compile()
    return nc


def _get_nc(shift: float) -> "bass.Bass":
    if shift not in _nc_cache:
        _nc_cache[shift] = _build(shift)
    return _nc_cache[shift]


def run(inputs: dict, trace: bool = False):
    m = np.asarray(inputs["modality_features"], dtype=np.float32)
    s = np.asarray(inputs["sequence_features"], dtype=np.float32)
    scale = float(np.asarray(inputs["logit_scale"], dtype=np.float32))
    assert m.shape == (B, D) and s.shape == (B, D)

    shift = float(6.0 * abs(scale) * math.sqrt(D))
    nc = _get_nc(shift)

    ms = m * np.float32(scale)
    sT_full = np.ascontiguousarray(s.T).astype(BF16)

    in_maps = []
    for c in range(NCORES):
        r = slice(c * ROWS, (c + 1) * ROWS)
        in_maps.append(
            {
                "mT": np.ascontiguousarray(ms[r].T).astype(BF16),
                "sT": sT_full,
                "mnat": np.ascontiguousarray(ms[r]),
                "snat": np.ascontiguousarray(s[r]),
            }
        )

    res = run_bass_kernel_spmd(nc, in_maps, list(range(NCORES)), trace=trace)

    rowsum = np.concatenate(
        [r["rowsum"].T.reshape(-1) for r in res.results]
    ).astype(np.float64)
    diag = np.concatenate([r["diag"].T.reshape(-1) for r in res.results]).astype(
        np.float64
    )
    colsum = np.zeros(B, dtype=np.float64)
    for r in res.results:
        colsum += r["colsum"].reshape(-1).astype(np.float64)

    rowlse = shift + np.log(rowsum)
    collse = shift + np.log(colsum)
    loss = np.mean(0.5 * (rowlse + collse) - diag)
    return np.asarray(loss, dtype=np.float32), res


def kernel(**inputs) -> np.ndarray:
    out, _ = run(inputs, trace=False)
    return out


# revision 9
# speedup vs baseline: 7290.6771x; 7290.6771x over previous
"""CLIP (InfoNCE) loss kernel for Trainium2, 8 NeuronCores.

loss = 0.5*(ce_m + ce_s) where
  ce_m = mean_i( LSE_j(l[i,:]) - l[i,i] ),  ce_s = mean_j( LSE_i(l[:,j]) - l[j,j] )
  l = logit_scale * (m @ s.T),  B=16384, D=256.

Strategy (data parallel on batch rows, 8 cores):
  - core c owns rows [c*2048, (c+1)*2048) of m; gets the FULL s.
  - host pre-scales m by logit_scale and pre-transposes both operands to
    [D, rows] bf16 so they feed the PE directly (lhsT / rhs layout).
  - per core, 16 m-tiles x 16 column-groups of [128, 1024] logits in PSUM
    (f32 accumulation over K=256 in two 128-chunks, 4 matmuls per group).
  - one ScalarE activation per group computes E = exp(l - SHIFT) (bf16);
    for half the groups the fused accum_out also emits the per-row partial
    sum (f32, on the un-rounded values); for the other half a DVE reduce
    of E does it — balancing ScalarE vs VectorE.
  - per-column sums of E accumulate in PSUM via ones-vector matmuls, four
    N=256 matmuls per group aimed at four different 32-column PE strips
    (tile_position) of ONE psum bank, so they execute concurrently.
  - diag l[i,i] is computed exactly in f32 as a row-dot of the
    natural-layout shards (DVE mul+reduce).
  - host merges the tiny per-core partials in float64:
      rowLSE = SHIFT + log(rowsum); colLSE = SHIFT + log(sum_c colsum_c)
      loss = mean(0.5*(rowLSE + colLSE) - diag)

SHIFT is a single global logsumexp shift. logits ~ N(0, (scale*sqrt(D))^2);
their max over B^2 samples is ~6 sigma, so SHIFT = 6*|scale|*sqrt(D) keeps
exp() in-range: no overflow, and anything that underflows f32 is ~e^-80
below the column max — far below f32 relative precision anyway.
"""

import math
from contextlib import ExitStack

import numpy as np
import ml_dtypes

import concourse.bacc as bacc
import concourse.bass as bass
import concourse.tile as tile
from concourse import mybir
from concourse.bass_utils import run_bass_kernel_spmd

BF16 = ml_dtypes.bfloat16

B = 16384
D = 256
NCORES = 8
ROWS = B // NCORES          # 2048 rows per core
P = 128
MT = ROWS // P              # 16 m-tiles
PN = 512                    # psum bank width (f32)
GW = 2                      # panels per exp-group -> [128, 1024] ACT ops
GN = B // (PN * GW)         # 16 column-groups
KC = D // P                 # 2 contraction chunks
NQ = 8                      # sT DMA chunks per k (early-start pipelining)
QW = B // NQ
CS = 2                      # column-sum split per panel (4 PE strips total)

f32 = mybir.dt.float32
bf16 = mybir.dt.bfloat16

_nc_cache: dict[float, "bass.Bass"] = {}


def _build(shift: float) -> "bass.Bass":
    nc = bacc.Bacc(trn_type="TRN2")

    mT_d = nc.dram_tensor("mT", [D, ROWS], bf16, kind="ExternalInput")
    sT_d = nc.dram_tensor("sT", [D, B], bf16, kind="ExternalInput")
    mnat_d = nc.dram_tensor("mnat", [ROWS, D], f32, kind="ExternalInput")
    snat_d = nc.dram_tensor("snat", [ROWS, D], f32, kind="ExternalInput")

    rowsum_d = nc.dram_tensor("rowsum", [P, MT], f32, kind="ExternalOutput")
    diag_d = nc.dram_tensor("diag", [P, MT], f32, kind="ExternalOutput")
    colsum_d = nc.dram_tensor("colsum", [GW, GN * PN], f32, kind="ExternalOutput")

    nstrips = GW * CS
    w = PN // CS

    with ExitStack() as ctx:
        tc = ctx.enter_context(tile.TileContext(nc))
        singles = ctx.enter_context(tc.tile_pool(name="singles", bufs=1))
        epool = ctx.enter_context(tc.tile_pool(name="epool", bufs=6))
        diagpool = ctx.enter_context(tc.tile_pool(name="diagpool", bufs=4))
        mainps = ctx.enter_context(tc.tile_pool(name="mainps", bufs=3, space="PSUM"))
        colps = ctx.enter_context(tc.tile_pool(name="colps", bufs=2, space="PSUM"))

        mT_sb = singles.tile([P, KC, ROWS], bf16, tag="mT")
        for k in range(KC):
            nc.sync.dma_start(out=mT_sb[:, k, :], in_=mT_d[k * P : (k + 1) * P, :])
        sT_sb = [
            [
                singles.tile([P, QW], bf16, name=f"sT_{k}_{q}", tag=f"sT_{k}_{q}")
                for q in range(NQ)
            ]
            for k in range(KC)
        ]
        # q-major order so the first column-group's two k-chunks land first
        for q in range(NQ):
            for k in range(KC):
                nc.sync.dma_start(
                    out=sT_sb[k][q],
                    in_=sT_d[k * P : (k + 1) * P, q * QW : (q + 1) * QW],
                )

        ones = singles.tile([P, 1], bf16, tag="ones")
        nc.vector.memset(ones, 1.0)
        negshift = singles.tile([P, 1], f32, tag="negshift")
        nc.vector.memset(negshift, -shift)

        rowsums_sb = singles.tile([P, MT * GN], f32, tag="rowsums")
        rowfinal = singles.tile([P, MT], f32, tag="rowfinal")
        colsum_sb = [
            singles.tile([1, GN * PN], f32, name=f"colsum_{i}", tag=f"colsum_{i}")
            for i in range(GW)
        ]
        diagfinal = singles.tile([P, MT], f32, tag="diagfinal")

        for g in range(GN):
            colpsum = colps.tile([32 * (nstrips - 1) + 1, PN], f32)  # one bank
            for mt in range(MT):
                ps = mainps.tile([P, GW * PN], f32)  # 2 banks
                for k in range(KC):
                    for sub in range(GW):
                        nt = g * GW + sub
                        q, j = divmod(nt, QW // PN)
                        nc.tensor.matmul(
                            ps[:, sub * PN : (sub + 1) * PN],
                            lhsT=mT_sb[:, k, mt * P : (mt + 1) * P],
                            rhs=sT_sb[k][q][:, j * PN : (j + 1) * PN],
                            start=(k == 0),
                            stop=(k == KC - 1),
                        )
                e = epool.tile([P, GW * PN], bf16)
                slot = mt * GN + g
                if (mt + g) % 2 == 0:
                    nc.scalar.activation(
                        e, ps, mybir.ActivationFunctionType.Exp,
                        bias=negshift[:, 0:1], scale=1.0,
                        accum_out=rowsums_sb[:, slot : slot + 1],
                    )
                else:
                    nc.scalar.activation(
                        e, ps, mybir.ActivationFunctionType.Exp,
                        bias=negshift[:, 0:1], scale=1.0,
                    )
                    nc.vector.reduce_sum(
                        rowsums_sb[:, slot : slot + 1], e,
                        axis=mybir.AxisListType.X,
                    )
                for sub in range(GW):
                    for ci in range(CS):
                        strip = sub * CS + ci
                        nc.tensor.matmul(
                            colpsum[32 * strip : 32 * strip + 1, 0:w],
                            lhsT=ones,
                            rhs=e[:, sub * PN + ci * w : sub * PN + (ci + 1) * w],
                            start=(mt == 0),
                            stop=(mt == MT - 1),
                            tile_position=(0, 32 * strip),
                        )
            for sub in range(GW):
                for ci in range(CS):
                    strip = sub * CS + ci
                    nc.vector.tensor_copy(
                        out=colsum_sb[sub][
                            :, g * PN + ci * w : g * PN + (ci + 1) * w
                        ],
                        in_=colpsum[32 * strip : 32 * strip + 1, 0:w],
                    )

        # diag + final row reduction emitted last (lowest scheduler priority;
        # DVE/DMA fill gaps while PE/ACT stream)
        for mt in range(MT):
            mn = diagpool.tile([P, D], f32, tag="mn")
            sn = diagpool.tile([P, D], f32, tag="sn")
            prod = diagpool.tile([P, D], f32, tag="prod")
            nc.sync.dma_start(out=mn, in_=mnat_d[mt * P : (mt + 1) * P, :])
            nc.sync.dma_start(out=sn, in_=snat_d[mt * P : (mt + 1) * P, :])
            # (tensor_tensor_reduce compiles but faults on this HW/runtime
            # combo — use plain mul + reduce instead)
            nc.vector.tensor_mul(prod, mn, sn)
            nc.vector.reduce_sum(
                diagfinal[:, mt : mt + 1], prod, axis=mybir.AxisListType.X
            )

        for mt in range(MT):
            nc.vector.reduce_sum(
                rowfinal[:, mt : mt + 1],
                rowsums_sb[:, mt * GN : (mt + 1) * GN],
                axis=mybir.AxisListType.X,
            )

        nc.sync.dma_start(out=rowsum_d[:, :], in_=rowfinal)
        nc.sync.dma_start(out=diag_d[:, :], in_=diagfinal)
        for sub in range(GW):
            nc.sync.dma_start(out=colsum_d[sub : sub + 1, :], in_=colsum_sb[sub])

    nc.compile()
    return nc


def _get_nc(shift: float) -> "bass.Bass":
    if shift not in _nc_cache:
        _nc_cache[shift] = _build(shift)
    return _nc_cache[shift]


def run(inputs: dict, trace: bool = False):
    m = np.asarray(inputs["modality_features"], dtype=np.float32)
    s = np.asarray(inputs["sequence_features"], dtype=np.float32)
    scale = float(np.asarray(inputs["logit_scale"], dtype=np.float32))
    assert m.shape == (B, D) and s.shape == (B, D)

    shift = float(6.0 * abs(scale) * math.sqrt(D))
    nc = _get_nc(shift)

    ms = m * np.float32(scale)
    sT_full = np.ascontiguousarray(s.T).astype(BF16)

    in_maps = []
    for c in range(NCORES):
        r = slice(c * ROWS, (c + 1) * ROWS)
        in_maps.append(
            {
                "mT": np.ascontiguousarray(ms[r].T).astype(BF16),
                "sT": sT_full,
                "mnat": np.ascontiguousarray(ms[r]),
                "snat": np.ascontiguousarray(s[r]),
            }
        )

    res = run_bass_kernel_spmd(nc, in_maps, list(range(NCORES)), trace=trace)

    rowsum = np.concatenate(
        [r["rowsum"].T.reshape(-1) for r in res.results]
    ).astype(np.float64)
    diag = np.concatenate([r["diag"].T.reshape(-1) for r in res.results]).astype(
        np.float64
    )
    colsum = np.zeros(B, dtype=np.float64)
    for r in res.results:
        # colsum_d[sub, g*PN + j] holds column g*(GW*PN) + sub*PN + j
        arr = r["colsum"].astype(np.float64)
        colsum += arr.reshape(GW, GN, PN).transpose(1, 0, 2).reshape(B)

    rowlse = shift + np.log(rowsum)
    collse = shift + np.log(colsum)
    loss = np.mean(0.5 * (rowlse + collse) - diag)
    return np.asarray(loss, dtype=np.float32), res


def kernel(**inputs) -> np.ndarray:
    out, _ = run(inputs, trace=False)
    return out
